# revision 27
# baseline (speedup 1.0000x reference)
"""ADDS loss kernel for Trainium2, SPMD over 8 NeuronCores.

Problem: pred = model_points @ pred_R^T + pred_t (per batch), gt likewise;
d2[b,n,m] = ||pred[b,n] - gt[b,m]||^2; out = mean_{b,n} sqrt(max(min_m d2, 0)).

Sharding: data-parallel over batch B=32 -> 4 batches per core, one 5-row
operand group per batch at partition base 32*b:
  pred_stuff rows = [-2*p_x, -2*p_y, -2*p_z, pn2, 1]
  gt_stuff   rows = [g_x, g_y, g_z, 1, gn2]
so a K=5 matmul yields d2[n, m] = -2 p.g + pn2[n] + gn2[m] directly in PSUM.

Reduction: per (n_chunk, batch) the 4 m-chunk PSUM tiles are consumed by
ACT (bf16 convert of 2 tiles to SBUF) + DVE (2 chained tensor_tensor_scan
running-mins over (PSUM tile, SBUF tile) pairs) -> [128,1] min with no
reduce tail. Then clamp, sqrt, sum. Host sums the 8x[128,1] partials.
"""

import numpy as np

import concourse.bass as bass
import concourse.bacc as bacc_mod
import concourse.mybir as mybir
from concourse.tile import TileContext
from concourse.bass_utils import run_bass_kernel_spmd

B = 32
N = 2048
NCORES = 8
BPC = B // NCORES  # batches per core = 4
FP32 = mybir.dt.float32
BF16 = mybir.dt.bfloat16
AF = mybir.ActivationFunctionType
OP = mybir.AluOpType

# float32r streams at 1 col/cycle for moving dims >= 256 (vs 1/4 for fp32)
USE_F32R = True
BIG_INIT = 1.0e30

# tuning knobs (overridable per-build)
DEFAULT_CFG = dict(
    tree_depth=3,  # TT-min tree levels before the final reduce
    warmup=0,      # optional PE warm-up burst (not beneficial; keep 0)
    wide=False,    # one [128,2048] psum tile/group vs two [128,1024]
    nd_mod=7,      # 1-in-nd_mod groups go DVE-direct (ACT/DVE balance)
    sbf_bufs=3,
    work_bufs=2,
)


def _mm_dt(ap):
    return ap.bitcast(mybir.dt.float32r) if USE_F32R else ap


def build_kernel(**cfg_over):
    cfg = dict(DEFAULT_CFG)
    cfg.update(cfg_over)
    nc = bacc_mod.Bacc()

    F32R = mybir.dt.float32r
    pointsT_ext = nc.declare_dram_parameter("pointsT", [3, N], F32R, isOutput=False)
    Rp_ext = nc.declare_dram_parameter("Rp", [3, 128], F32R, isOutput=False)
    Rg_ext = nc.declare_dram_parameter("Rg", [3, 128], F32R, isOutput=False)
    biasp_ext = nc.declare_dram_parameter("biasp", [128, 1], FP32, isOutput=False)
    biasg_ext = nc.declare_dram_parameter("biasg", [128, 1], FP32, isOutput=False)
    onesp_ext = nc.declare_dram_parameter("onesp", [128, 128], F32R, isOutput=False)
    onesg_ext = nc.declare_dram_parameter("onesg", [128, 128], F32R, isOutput=False)
    out_ext = nc.declare_dram_parameter("out", [128, 1], FP32, isOutput=True)

    with TileContext(nc) as tc:
        with (
            tc.tile_pool(name="persist", bufs=1) as persist,
            tc.tile_pool(name="work", bufs=cfg["work_bufs"]) as work,
            tc.tile_pool(name="sbf", bufs=cfg["sbf_bufs"]) as sbf,
            tc.tile_pool(name="ps", bufs=(2 if cfg["wide"] else 4), space="PSUM") as ps,
        ):
            # ---- load inputs ----
            def load(ext, shape, nm, dt=FP32):
                t = persist.tile(shape, dt, tag=nm, name=nm)
                nc.sync.dma_start(out=t[:, :], in_=ext[:, :])
                return t

            F32R = mybir.dt.float32r
            pointsT = load(pointsT_ext, [3, N], "pointsT_sb", F32R)
            Rsb = {}
            biassb = {}
            onessb = {}
            for side, (R_ext, b_ext, o_ext) in (
                ("p", (Rp_ext, biasp_ext, onesp_ext)),
                ("g", (Rg_ext, biasg_ext, onesg_ext)),
            ):
                Rsb[side] = load(R_ext, [3, 128], f"R{side}_sb", F32R)
                biassb[side] = load(b_ext, [128, 1], f"bias{side}_sb")
                onessb[side] = load(o_ext, [128, 128], f"ones{side}_sb", F32R)

            # ---- Phase A: build stuff_p / stuff_g (all f32r) ----
            # Inputs arrive pre-rounded to f32r precision from the host.
            # gt side first: phase B needs all gt m-chunks but only the
            # first pred n-chunks to start.
            stuff = {}
            for side, scale in (("g", 1.0), ("p", -2.0)):
                stp = persist.tile([128, N], F32R, tag=f"stp{side}", name=f"stp{side}_sb")
                sq = work.tile([128, N], F32R, tag=f"sq{side}", name=f"sq{side}")
                for c in range(N // 512):
                    cs = slice(c * 512, (c + 1) * 512)
                    T = ps.tile([128, 512], FP32, tag="psb", name="psb")
                    nc.tensor.matmul(
                        T[:, :], Rsb[side][:, :], pointsT[:, cs],
                        start=True, stop=True,
                    )
                    # stuff = scale*transform + bias (rounded to f32r)
                    nc.scalar.activation(
                        stp[:, cs], T[:, :], AF.Identity,
                        bias=biassb[side][:, :], scale=scale,
                    )
                    # square on DVE (f32r out feeds the norms matmul)
                    nc.vector.tensor_tensor(
                        sq[:, cs], stp[:, cs], stp[:, cs], op=OP.mult
                    )
                    N_ps = ps.tile([128, 512], FP32, tag="psb", name="psb")
                    nc.tensor.matmul(
                        N_ps[:, :], onessb[side][:, :], sq[:, cs],
                        start=True, stop=True,
                    )
                    # fold norm rows into stuff (other rows of N_ps are 0)
                    nc.vector.tensor_tensor(
                        stp[:, cs], stp[:, cs], N_ps[:, :], op=OP.add
                    )
                stuff[side] = stp

            # ---- PE warm-up: a dense burst of junk matmuls keeps the
            # HAM activity monitor busy so the PE clock ramps to 2.4 GHz
            # before (and into) the main loop. Uses one pooled PSUM slot,
            # released after a single cheap consume.
            if cfg["warmup"]:
                wtile = ps.tile([128, 512], FP32, tag="psb", name="warmtile")
                for _w in range(cfg["warmup"]):
                    nc.tensor.matmul(
                        wtile[:, :],
                        stuff["p"][0:5, 0:128],
                        stuff["g"][0:5, 0:512],
                        start=True,
                        stop=True,
                    )
                wres = persist.tile([128, 1], FP32, tag="wres", name="wres")
                nc.vector.tensor_reduce(
                    wres[:, :], wtile[:, 0:64], axis=mybir.AxisListType.X, op=OP.min
                )
                wjunk = nc.dram_tensor("warm_junk", [128, 1], FP32)
                nc.sync.dma_start(out=wjunk[:, :], in_=wres[:, :])

            # ---- Phase B: main loop ----
            # Per (nch, b) group: 2048 m-values in PSUM ([128,2048] as one
            # tile, or two [128,1024] tiles), 4 f32r matmuls. Tree groups:
            # ACT bf16-converts to S, DVE runs a 2x TT-min tree; direct
            # groups (1 in nd_mod): DVE reduce_min straight from PSUM.
            # Clamped mins collect into roots; sqrt batched at the end.
            roots = persist.tile([128, 16 * BPC], FP32, tag="roots", name="roots")
            for nch in range(16):
                min4 = work.tile([128, BPC], FP32, tag="min4", name="min4")
                for b in range(BPC):
                    g = nch * BPC + b
                    lhs = stuff["p"][32 * b : 32 * b + 5, nch * 128 : (nch + 1) * 128]
                    direct = (g % cfg["nd_mod"]) == (cfg["nd_mod"] - 1)
                    if cfg["wide"]:
                        P = ps.tile([128, 2048], FP32, tag="psb", name="psb")
                        halves = [P[:, 0:1024], P[:, 1024:2048]]
                        for mc in range(4):
                            nc.tensor.matmul(
                                P[:, mc * 512 : (mc + 1) * 512],
                                lhs,
                                stuff["g"][32 * b : 32 * b + 5, mc * 512 : (mc + 1) * 512],
                                start=True,
                                stop=True,
                                tile_position=(32 * b, 0),
                            )
                        wholes = [P[:, :]]
                    else:
                        halves = []
                        for h in range(2):
                            P = ps.tile([128, 1024], FP32, tag="psb", name="psb")
                            halves.append(P[:, :])
                            for mc in range(2):
                                m0 = (2 * h + mc) * 512
                                nc.tensor.matmul(
                                    P[:, mc * 512 : (mc + 1) * 512],
                                    lhs,
                                    stuff["g"][32 * b : 32 * b + 5, m0 : m0 + 512],
                                    start=True,
                                    stop=True,
                                    tile_position=(32 * b, 0),
                                )
                        wholes = None
                    if direct:
                        if cfg["wide"]:
                            nc.vector.tensor_reduce(
                                min4[:, b : b + 1], wholes[0],
                                axis=mybir.AxisListType.X, op=OP.min,
                            )
                        else:
                            m2 = work.tile([128, 2], FP32, tag="m2", name="m2")
                            for h in range(2):
                                nc.vector.tensor_reduce(
                                    m2[:, h : h + 1], halves[h],
                                    axis=mybir.AxisListType.X, op=OP.min,
                                )
                            nc.vector.tensor_reduce(
                                min4[:, b : b + 1], m2[:, :],
                                axis=mybir.AxisListType.X, op=OP.min,
                            )
                    else:
                        S = sbf.tile([128, 2048], BF16, tag="S", name="S")
                        if cfg["wide"]:
                            nc.scalar.copy(S[:, :], wholes[0])
                        else:
                            for h in range(2):
                                nc.scalar.copy(
                                    S[:, h * 1024 : (h + 1) * 1024], halves[h]
                                )
                        u1 = sbf.tile([128, 1024], BF16, tag="u1", name="u1")
                        nc.vector.tensor_tensor(
                            u1[:, :], S[:, 0:1024], S[:, 1024:2048], op=OP.min
                        )
                        last = u1
                        width = 512
                        for lvl in range(cfg["tree_depth"] - 1):
                            nxt = sbf.tile(
                                [128, width], BF16, tag=f"u{lvl+2}", name=f"u{lvl+2}"
                            )
                            nc.vector.tensor_tensor(
                                nxt[:, :], last[:, 0:width], last[:, width : 2 * width],
                                op=OP.min,
                            )
                            last = nxt
                            width //= 2
                        nc.vector.tensor_reduce(
                            min4[:, b : b + 1], last[:, :],
                            axis=mybir.AxisListType.X, op=OP.min,
                        )
                # clamp at 0 into roots (sqrt batched at the end)
                nc.vector.tensor_scalar(
                    roots[:, nch * BPC : (nch + 1) * BPC], min4[:, :], 0.0, None,
                    op0=OP.max,
                )

            # ---- final: sqrt then sum over the 64 roots columns ----
            roots2 = persist.tile([128, 16 * BPC], FP32, tag="roots2", name="roots2")
            nc.scalar.activation(roots2[:, :], roots[:, :], AF.Sqrt)
            acc = persist.tile([128, 1], FP32, tag="acc", name="acc")
            nc.vector.tensor_reduce(
                acc[:, :], roots2[:, :], axis=mybir.AxisListType.X, op=OP.add
            )
            nc.sync.dma_start(out=out_ext[:, :], in_=acc[:, :])

    nc.compile()
    return nc


_NC_CACHE = None


def _get_nc():
    global _NC_CACHE
    if _NC_CACHE is None:
        _NC_CACHE = build_kernel()
    return _NC_CACHE


def _round_f32r(x):
    """Round fp32 to float32r precision (12-bit mantissa, round-to-nearest)."""
    xi = np.ascontiguousarray(x, np.float32).view(np.uint32)
    drop = 11
    bias = ((xi >> drop) & 1) + ((1 << (drop - 1)) - 1)
    mask = np.uint32(0xFFFFFFFF ^ ((1 << drop) - 1))
    return ((xi + bias) & mask).view(np.float32)


def make_in_maps(pred_R, pred_t, gt_R, gt_t, model_points):
    pointsT = _round_f32r(np.ascontiguousarray(model_points.T.astype(np.float32)))  # [3, N]
    in_maps = []
    for core in range(NCORES):
        Rp = np.zeros((3, 128), np.float32)
        Rg = np.zeros((3, 128), np.float32)
        biasp = np.zeros((128, 1), np.float32)
        biasg = np.zeros((128, 1), np.float32)
        onesp = np.zeros((128, 128), np.float32)
        onesg = np.zeros((128, 128), np.float32)
        for b in range(BPC):
            gb = core * BPC + b
            base = 32 * b
            Rp[:, base : base + 3] = _round_f32r(pred_R[gb].T)
            Rg[:, base : base + 3] = _round_f32r(gt_R[gb].T)
            biasp[base : base + 3, 0] = -2.0 * pred_t[gb]
            biasg[base : base + 3, 0] = gt_t[gb]
            biasp[base + 4, 0] = 1.0  # pred ones row
            biasg[base + 3, 0] = 1.0  # gt ones row
            # pred pn2 at base+3 (0.25 * sum a^2, a = -2p); gt gn2 at base+4
            onesp[base : base + 3, base + 3] = 0.25
            onesg[base : base + 3, base + 4] = 1.0
        in_maps.append(
            {
                "pointsT": pointsT,
                "Rp": Rp,
                "Rg": Rg,
                "biasp": biasp,
                "biasg": biasg,
                "onesp": onesp,
                "onesg": onesg,
            }
        )
    return in_maps


def kernel(pred_R, pred_t, gt_R, gt_t, model_points):
    pred_R = np.asarray(pred_R, np.float32)
    pred_t = np.asarray(pred_t, np.float32)
    gt_R = np.asarray(gt_R, np.float32)
    gt_t = np.asarray(gt_t, np.float32)
    model_points = np.asarray(model_points, np.float32)

    nc = _get_nc()
    in_maps = make_in_maps(pred_R, pred_t, gt_R, gt_t, model_points)
    res = run_bass_kernel_spmd(nc, in_maps, core_ids=list(range(NCORES)))
    total = np.float64(0.0)
    for r in res.results:
        total += np.asarray(r["out"], np.float64).sum()
    return np.float32(total / (B * N))
